# revision 14
# baseline (speedup 1.0000x reference)
"""Multi-head attention (B=4, S=2048, E=1024, H=16) on 8 NeuronCores.

Sharding: data-parallel over (batch, query-half): core c handles batch c//2,
query rows (c%2)*1024:(c%2+1)*1024, with the full K/V rows of that batch.
No collectives; output slices are disjoint and concatenated on host.
Activations are distributed to each core pre-transposed ([E, rows] layout) so
the device kernel needs no transpose passes; weights ship in natural layout.

Per-core program (all matmuls fp32r = full-rate ~fp32-precision mode):
  PV  V = (vinT^T Wv) in [k, e] layout with an appended ones column (V_aug);
      bv is folded out: P@(V+1*bv)/denom = P@V/denom + bv.
  PK  K^T = Wk^T kinT + bk   ([e, k] layout, resident)
  PQ  Q^T = Wq^T qinT + bq   ([e, q] layout, resident)
  PA  per (head, q-chunk), streaming over k-tile pairs:
        S^T  = K_h^T-tiles @ Q_h^T          (PSUM, k on partitions)
        expS = Exp(S^T * 1/8)               (no max subtraction: scores ~N(0,1))
        ctx_unT/denom = V_aug^T @ expS      (PSUM accumulation over k tiles)
        ctx^T = ctx_unT * bcast(1/denom)    -> DRAM staging (e, q)
  PO  out = ctx^T-tiles^T @ Wo + ones*(bv@Wo + bo)  (rank-1 PSUM bias update)
"""

import os
import sys

for _p in ("/opt/trn_rl_repo", os.path.expanduser("~/.axon_site/_ro/trn_rl_repo")):
    if os.path.isdir(_p) and _p not in sys.path:
        sys.path.append(_p)

import numpy as np

import concourse.bass as bass
import concourse.tile as tile
from concourse import bacc, mybir
from concourse.bass_utils import run_bass_kernel_spmd

E = 1024
H = 16
D = 64
B = 4
S = 2048
P = 128
RQ = 1024  # query rows per core
RK = 2048  # kv rows per core
F32 = mybir.dt.float32
F32R = mybir.dt.float32r
N_CORES = 8

_CACHE = {}
_LAST_RESULTS = None


def _build_program():
    nc = bacc.Bacc("TRN2", target_bir_lowering=False, debug=False, num_devices=N_CORES)

    # Activations pre-transposed on host: [E, rows]
    qinT_d = nc.dram_tensor("qinT", [E, RQ], F32, kind="ExternalInput").ap()
    kinT_d = nc.dram_tensor("kinT", [E, RK], F32, kind="ExternalInput").ap()
    vinT_d = nc.dram_tensor("vinT", [E, RK], F32, kind="ExternalInput").ap()
    Wq = nc.dram_tensor("Wq", [E, E], F32, kind="ExternalInput").ap()
    Wk = nc.dram_tensor("Wk", [E, E], F32, kind="ExternalInput").ap()
    Wv = nc.dram_tensor("Wv", [E, E], F32, kind="ExternalInput").ap()
    Wo = nc.dram_tensor("Wo", [E, E], F32, kind="ExternalInput").ap()
    bq = nc.dram_tensor("bq", [E], F32, kind="ExternalInput").ap()
    bk = nc.dram_tensor("bk", [E], F32, kind="ExternalInput").ap()
    bv = nc.dram_tensor("bv", [E], F32, kind="ExternalInput").ap()
    bo = nc.dram_tensor("bo", [E], F32, kind="ExternalInput").ap()
    out = nc.dram_tensor("out", [RQ, E], F32, kind="ExternalOutput").ap()
    ctxT_d = nc.dram_tensor("ctxT_stage", [E, RQ], F32).ap()

    ET = E // P  # 8 e-tiles
    NKT = RK // P  # 16 k-tiles

    with tile.TileContext(nc) as tc:
        with (
            tc.tile_pool(name="const", bufs=1) as const,
            tc.tile_pool(name="persist", bufs=1) as persist,
        ):
            bq_sb = const.tile([P, ET], F32)
            nc.sync.dma_start(out=bq_sb[:], in_=bq.rearrange("(t p) -> p t", p=P))
            bk_sb = const.tile([P, ET], F32)
            nc.sync.dma_start(out=bk_sb[:], in_=bk.rearrange("(t p) -> p t", p=P))
            bv_sb = const.tile([P, ET], F32R)
            nc.sync.dma_start(
                out=bv_sb[:], in_=bv.rearrange("(t p) -> p t", p=P).bitcast(F32R)
            )
            bo_sb = const.tile([1, E], F32)
            nc.sync.dma_start(out=bo_sb[:], in_=bo.rearrange("(p e) -> p e", p=1))
            ones_scr = const.tile([P, 2 * P], F32)
            nc.vector.memset(ones_scr[:], 1.0)
            ones1 = const.tile([1, P], F32R)
            nc.vector.tensor_copy(ones1[:], ones_scr[0:1, 0:P])

            # V_aug: [k-in-tile, k-tile, head, d+1] ; 66.5KB/part
            v_sb = persist.tile([P, NKT, H, D + 1], F32R)
            nc.vector.tensor_copy(
                v_sb[:, :, :, D],
                ones_scr[:].rearrange("p (a b) -> p a b", a=NKT),
            )

            # ---- PV: V projection (all 16 heads) ---------------------------
            with (
                tc.tile_pool(name="pv_in", bufs=2) as in_pool,
                tc.tile_pool(name="pv_w", bufs=1) as w_pool,
                tc.tile_pool(name="pv_ps", bufs=3, space="PSUM") as mm_psum,
            ):
                wv_sb = w_pool.tile([P, ET, E], F32R)
                for ke in range(ET):
                    nc.sync.dma_start(
                        out=wv_sb[:, ke, :], in_=Wv[ke * P : (ke + 1) * P, :].bitcast(F32R)
                    )
                for half in range(2):
                    k0 = half * (RK // 2)
                    vinT = in_pool.tile([P, ET, RK // 2], F32R, tag="vinT")
                    for ke in range(ET):
                        nc.sync.dma_start(
                            out=vinT[:, ke, :],
                            in_=vinT_d[ke * P : (ke + 1) * P, k0 : k0 + RK // 2].bitcast(F32R),
                        )
                    for kt in range(RK // 2 // P):  # 8 k-tiles per half
                        for ch in range(2):  # e_out chunks of 512 = 8 heads
                            ps = mm_psum.tile([P, 512], F32, tag="pvmm")
                            for ke in range(ET):
                                nc.tensor.matmul(
                                    ps[:],
                                    lhsT=vinT[:, ke, kt * P : (kt + 1) * P],
                                    rhs=wv_sb[:, ke, ch * 512 : (ch + 1) * 512],
                                    start=(ke == 0),
                                    stop=(ke == ET - 1),
                                )
                            if (kt + ch) % 2 == 0:
                                nc.vector.tensor_copy(
                                    v_sb[:, half * 8 + kt, ch * 8 : (ch + 1) * 8, 0:D],
                                    ps[:].rearrange("p (h d) -> p h d", h=8),
                                )
                            else:
                                nc.scalar.copy(
                                    v_sb[:, half * 8 + kt, ch * 8 : (ch + 1) * 8, 0:D],
                                    ps[:].rearrange("p (h d) -> p h d", h=8),
                                )

            # ---- PK: K^T = Wk^T @ kinT + bk --------------------------------
            persist_k_cm = tc.tile_pool(name="persist_k", bufs=1)
            persist_k = persist_k_cm.__enter__()
            kT = persist_k.tile([P, ET, RK], F32R)  # 64KB/part  [e, k]
            with (
                tc.tile_pool(name="pk_in", bufs=2) as in_pool,
                tc.tile_pool(name="pk_w", bufs=1) as w_pool,
                tc.tile_pool(name="pk_ps", bufs=3, space="PSUM") as mm_psum,
            ):
                wk_sb = w_pool.tile([P, ET, E], F32R)
                for ke in range(ET):
                    nc.sync.dma_start(
                        out=wk_sb[:, ke, :], in_=Wk[ke * P : (ke + 1) * P, :].bitcast(F32R)
                    )
                for qtr in range(4):
                    k0 = qtr * 512
                    kinT = in_pool.tile([P, ET, 512], F32R, tag="kinT")
                    for ke in range(ET):
                        nc.sync.dma_start(
                            out=kinT[:, ke, :],
                            in_=kinT_d[ke * P : (ke + 1) * P, k0 : k0 + 512].bitcast(F32R),
                        )
                    for et in range(ET):
                        ps = mm_psum.tile([P, 512], F32, tag="pkmm")
                        for ke in range(ET):
                            nc.tensor.matmul(
                                ps[:],
                                lhsT=wk_sb[:, ke, et * P : (et + 1) * P],
                                rhs=kinT[:, ke, :],
                                start=(ke == 0),
                                stop=(ke == ET - 1),
                            )
                        if et % 2 == 0:
                            nc.vector.tensor_scalar_add(
                                kT[:, et, k0 : k0 + 512], ps[:], bk_sb[:, et : et + 1]
                            )
                        else:
                            nc.scalar.activation(
                                kT[:, et, k0 : k0 + 512],
                                ps[:],
                                mybir.ActivationFunctionType.Identity,
                                bias=bk_sb[:, et : et + 1],
                            )

            # ---- PQ: Q^T = Wq^T @ qinT + bq --------------------------------
            persist_q_cm = tc.tile_pool(name="persist_q", bufs=1)
            persist_q = persist_q_cm.__enter__()
            qT = persist_q.tile([P, ET, RQ], F32R)  # 32KB/part  [e, q]
            with (
                tc.tile_pool(name="pq_in", bufs=2) as in_pool,
                tc.tile_pool(name="pq_w", bufs=2) as w_pool,
                tc.tile_pool(name="pq_ps", bufs=3, space="PSUM") as mm_psum,
            ):
                for qtr in range(4):
                    q0 = qtr * 256
                    qinT = in_pool.tile([P, ET, 256], F32R, tag="qinT")
                    for ke in range(ET):
                        nc.sync.dma_start(
                            out=qinT[:, ke, :],
                            in_=qinT_d[ke * P : (ke + 1) * P, q0 : q0 + 256].bitcast(F32R),
                        )
                    for et in range(ET):
                        wq_t = w_pool.tile([P, ET, P], F32R, tag="wq")
                        nc.sync.dma_start(
                            out=wq_t[:],
                            in_=Wq[:, et * P : (et + 1) * P]
                            .rearrange("(ke p) c -> p ke c", p=P)
                            .bitcast(F32R),
                        )
                        ps = mm_psum.tile([P, 256], F32, tag="pqmm")
                        for ke in range(ET):
                            nc.tensor.matmul(
                                ps[:],
                                lhsT=wq_t[:, ke, :],
                                rhs=qinT[:, ke, :],
                                start=(ke == 0),
                                stop=(ke == ET - 1),
                            )
                        if et % 2 == 0:
                            nc.vector.tensor_scalar_add(
                                qT[:, et, q0 : q0 + 256], ps[:], bq_sb[:, et : et + 1]
                            )
                        else:
                            nc.scalar.activation(
                                qT[:, et, q0 : q0 + 256],
                                ps[:],
                                mybir.ActivationFunctionType.Identity,
                                bias=bq_sb[:, et : et + 1],
                            )

            # ---- PA: attention, all 16 heads -------------------------------
            with (
                tc.tile_pool(name="pa_exp", bufs=3) as exp_pool,
                tc.tile_pool(name="pa_sps", bufs=2, space="PSUM") as s_psum,
                tc.tile_pool(name="pa_cps", bufs=2, space="PSUM") as c_psum,
                tc.tile_pool(name="pa_nrm", bufs=2) as nrm_pool,
            ):
                for h in range(H):
                    et_h = h // 2
                    p0 = (h % 2) * D
                    for qc in range(RQ // 512):
                        ctx_ps = c_psum.tile([D + 1, 512], F32, tag="ctx")
                        NK2 = NKT // 2
                        for kt2 in range(NK2):
                            s_ps = s_psum.tile([P, 2, 512], F32, tag="s")
                            for j in range(2):
                                nc.tensor.matmul(
                                    s_ps[:, j, :],
                                    lhsT=kT[
                                        p0 : p0 + D,
                                        et_h,
                                        (2 * kt2 + j) * P : (2 * kt2 + j + 1) * P,
                                    ],
                                    rhs=qT[p0 : p0 + D, et_h, qc * 512 : (qc + 1) * 512],
                                    start=True,
                                    stop=True,
                                )
                            exp_t = exp_pool.tile([P, 2, 512], F32R, tag="exp")
                            nc.scalar.activation(
                                exp_t[:],
                                s_ps[:],
                                mybir.ActivationFunctionType.Exp,
                                scale=0.125,
                            )
                            for j in range(2):
                                nc.tensor.matmul(
                                    ctx_ps[:],
                                    lhsT=v_sb[:, 2 * kt2 + j, h, :],
                                    rhs=exp_t[:, j, :],
                                    start=(kt2 == 0 and j == 0),
                                    stop=(kt2 == NK2 - 1 and j == 1),
                                )
                        recip = nrm_pool.tile([1, 512], F32, tag="recip")
                        nc.vector.reciprocal(recip[:], ctx_ps[D : D + 1, :])
                        rb = nrm_pool.tile([D, 512], F32, tag="rb")
                        nc.gpsimd.partition_broadcast(rb[:], recip[:])
                        ctxT_t = nrm_pool.tile([D, 512], F32, tag="ctxT")
                        nc.vector.tensor_mul(ctxT_t[:], ctx_ps[0:D, :], rb[:])
                        nc.sync.dma_start(
                            out=ctxT_d[h * D : (h + 1) * D, qc * 512 : (qc + 1) * 512],
                            in_=ctxT_t[:],
                        )

            persist_q_cm.__exit__(None, None, None)
            persist_k_cm.__exit__(None, None, None)

            # ---- PO: out = ctxT^T @ Wo + ones x (bv@Wo + bo) ---------------
            with (
                tc.tile_pool(name="po_w", bufs=1) as w_pool,
                tc.tile_pool(name="po_ctx", bufs=1) as ctx_pool,
                tc.tile_pool(name="po_row", bufs=1) as row_pool,
                tc.tile_pool(name="po_rps", bufs=2, space="PSUM") as r_psum,
                tc.tile_pool(name="po_ps", bufs=4, space="PSUM") as mm_psum,
                tc.tile_pool(name="po_out", bufs=3) as out_pool,
            ):
                wo_sb = w_pool.tile([P, ET, E], F32R)
                for ke in range(ET):
                    nc.sync.dma_start(
                        out=wo_sb[:, ke, :], in_=Wo[ke * P : (ke + 1) * P, :].bitcast(F32R)
                    )
                ctx_sb = ctx_pool.tile([P, ET, RQ], F32R)
                for et in range(ET):
                    nc.sync.dma_start(
                        out=ctx_sb[:, et, :], in_=ctxT_d[et * P : (et + 1) * P, :].bitcast(F32R)
                    )
                # row = bv @ Wo + bo   [1, E]
                row_sb = row_pool.tile([1, E], F32R)
                for ch in range(E // 512):
                    rps = r_psum.tile([1, 512], F32, tag="rowps")
                    for ke in range(ET):
                        nc.tensor.matmul(
                            rps[:],
                            lhsT=bv_sb[:, ke : ke + 1],
                            rhs=wo_sb[:, ke, ch * 512 : (ch + 1) * 512],
                            start=(ke == 0),
                            stop=(ke == ET - 1),
                        )
                    nc.vector.tensor_add(
                        row_sb[:, ch * 512 : (ch + 1) * 512],
                        rps[:],
                        bo_sb[:, ch * 512 : (ch + 1) * 512],
                    )
                for qt in range(RQ // P):
                    for ch in range(E // 512):
                        ps = mm_psum.tile([P, 512], F32, tag="pomm")
                        for ke in range(ET):
                            nc.tensor.matmul(
                                ps[:],
                                lhsT=ctx_sb[:, ke, qt * P : (qt + 1) * P],
                                rhs=wo_sb[:, ke, ch * 512 : (ch + 1) * 512],
                                start=(ke == 0),
                                stop=False,
                            )
                        nc.tensor.matmul(
                            ps[:],
                            lhsT=ones1[:],
                            rhs=row_sb[:, ch * 512 : (ch + 1) * 512],
                            start=False,
                            stop=True,
                        )
                        out_t = out_pool.tile([P, 512], F32, tag="out_t")
                        if (qt + ch) % 2 == 0:
                            nc.vector.tensor_copy(out_t[:], ps[:])
                        else:
                            nc.scalar.copy(out_t[:], ps[:])
                        nc.sync.dma_start(
                            out=out[qt * P : (qt + 1) * P, ch * 512 : (ch + 1) * 512],
                            in_=out_t[:],
                        )

    nc.compile()
    return nc


def _get_program():
    if "nc" not in _CACHE:
        _CACHE["nc"] = _build_program()
    return _CACHE["nc"]


def kernel(query, key, value, Wq, Wk, Wv, Wo, bq, bk, bv, bo):
    global _LAST_RESULTS
    query = np.asarray(query, dtype=np.float32)
    key = np.asarray(key, dtype=np.float32)
    value = np.asarray(value, dtype=np.float32)
    shared = {
        "Wq": np.ascontiguousarray(np.asarray(Wq, np.float32)),
        "Wk": np.ascontiguousarray(np.asarray(Wk, np.float32)),
        "Wv": np.ascontiguousarray(np.asarray(Wv, np.float32)),
        "Wo": np.ascontiguousarray(np.asarray(Wo, np.float32)),
        "bq": np.ascontiguousarray(np.asarray(bq, np.float32)),
        "bk": np.ascontiguousarray(np.asarray(bk, np.float32)),
        "bv": np.ascontiguousarray(np.asarray(bv, np.float32)),
        "bo": np.ascontiguousarray(np.asarray(bo, np.float32)),
    }
    in_maps = []
    for c in range(N_CORES):
        b, half = c // 2, c % 2
        in_maps.append(
            {
                "qinT": np.ascontiguousarray(query[b, half * RQ : (half + 1) * RQ, :].T),
                "kinT": np.ascontiguousarray(key[b].T),
                "vinT": np.ascontiguousarray(value[b].T),
                **shared,
            }
        )
    nc = _get_program()
    res = run_bass_kernel_spmd(nc, in_maps, list(range(N_CORES)))
    _LAST_RESULTS = res
    full = np.empty((B, S, E), dtype=np.float32)
    for c in range(N_CORES):
        b, half = c // 2, c % 2
        full[b, half * RQ : (half + 1) * RQ, :] = res.results[c]["out"]
    return full
